# revision 4
# baseline (speedup 1.0000x reference)
"""RBF kernel layer (retrieval_knn): out = exp(-||x - p||^2) for x [131072, 64]
against 512 prototypes, distributed data-parallel over 8 NeuronCores.

v2 design ([m,n] orientation, single fp16 GEMM, bf16 output):
  out[m, n] = exp(2*S[m, n] - p_sq[m]),  S = cross - x_sq/2
computed as ONE fp16 matmul per 512-column tile with K=66:
  lhsT = [p_t(64 rows); 1; 1]  (stationary, per 128-prototype tile)
  rhs  = [x_t(64 rows); nxsq_h; nxsq_l]  (x features + fp16 hi/lo of -x_sq/2)
The exact -p_sq[m] rides in the ACTIVATE's per-partition bias AP (f32), and
scale=2.0 turns PSUM S into exp(2S - p_sq) in one pass, emitted directly as
bf16 (halves output DMA bytes vs f32; host upconverts + transposes).

Engine budget per core (16384 points, 512 protos): ScalarE exp is the
bottleneck: 65536 elem/lane / 1.2GHz + 32 ACT overheads ~= 63us. DMA moves
2.2MB in + 16.8MB out ~= 50us. PE streams 128 matmuls of 512 cols ~= 27-55us
(HAM-dependent). DVE is idle (prototypes stationary => no lhsT rebuilds).
x_sq/p_sq are computed from the QUANTIZED fp16 inputs, so the kernel is the
exact RBF of (x16, p16): error ~ 2|x-p|*q_rms, small precisely where the
output is large.
"""

import numpy as np

# Problem constants (hardcoded per harness contract; kernel.py is self-contained)
N = 131072
D = 64
M = 512
GAMMA = 1.0
NCORES = 8
NSHARD = N // NCORES  # 16384
P = 128
K1 = D + 2  # contraction: 64 x rows + 2 (-x_sq/2 hi/lo vs ones) rows
MT = M // P  # 4 prototype tiles
NT = NSHARD // 512  # 32 column chunks of 512 points
OCHUNK = 4  # PSUM banks per ACTIVATE + output DMA (double-buffered 4+4)

_cache = {}


def _build_bass(nshard=NSHARD):
    import concourse.mybir as mybir
    import concourse.tile as tile
    from concourse import bacc

    f32 = mybir.dt.float32
    f16 = mybir.dt.float16
    bf16 = mybir.dt.bfloat16

    nc = bacc.Bacc(None, target_bir_lowering=False)
    # x pre-transposed on host: rows 0..63 = features, 64/65 = -x_sq/2 hi/lo
    xr_d = nc.dram_tensor("xr", [K1, nshard], f16, kind="ExternalInput")
    # prototypes transposed: rows 0..63 = features, 64/65 = ones
    lhs_d = nc.dram_tensor("lhs", [K1, M], f16, kind="ExternalInput")
    # npsq[p, t] = -p_sq[t*128 + p] (f32, exact)
    npsq_d = nc.dram_tensor("npsq", [P, MT], f32, kind="ExternalInput")
    # output transposed: out_t[m, n]; host converts to [n, m] f32
    out_d = nc.dram_tensor("out", [M, nshard], bf16, kind="ExternalOutput")

    with tile.TileContext(nc) as tc:
        with (
            tc.tile_pool(name="singles", bufs=1) as singles,
            tc.tile_pool(name="outp", bufs=4) as outp,
            tc.tile_pool(name="ps_o", bufs=2, space="PSUM") as ps_o,
        ):
            lhs_sb = singles.tile([K1, M], f16)
            nc.sync.dma_start(lhs_sb[:], lhs_d[:])
            npsq_sb = singles.tile([P, MT], f32)
            nc.sync.dma_start(npsq_sb[:], npsq_d[:])

            # x stays resident in SBUF (2.2MB); ramped chunk sizes so the
            # first matmuls start ~10us earlier (early DMA rate is low).
            X_sb = singles.tile([K1, nshard], f16)
            pos = 0
            for ch in (512, 512, 1024, 2048, 4096, 4096, 4096):
                nc.sync.dma_start(
                    X_sb[:, pos : pos + ch], xr_d[:, pos : pos + ch]
                )
                pos += ch
            assert pos == nshard

            for mt in range(MT):
                lhs_ap = lhs_sb[:, mt * P : (mt + 1) * P]
                bias_ap = npsq_sb[:, mt : mt + 1]
                for c in range(NT):
                    k = c % OCHUNK
                    if k == 0:
                        psum = ps_o.tile([P, OCHUNK, 512], f32, tag="psum")
                        o_sb = outp.tile([P, OCHUNK, 512], bf16, tag="o")
                    nc.tensor.matmul(
                        psum[:, k, :],
                        lhs_ap,
                        X_sb[:, c * 512 : (c + 1) * 512],
                        start=True,
                        stop=True,
                    )
                    if k == OCHUNK - 1:
                        c0 = c - (OCHUNK - 1)
                        last = mt == MT - 1 and c == NT - 1
                        # out = exp(2*S - p_sq); the very last group is split
                        # in halves so the final output DMA tail is shorter
                        splits = (
                            [(0, 2), (2, 4)] if last else [(0, OCHUNK)]
                        )
                        for a, b in splits:
                            nc.scalar.activation(
                                o_sb[:, a:b, :],
                                psum[:, a:b, :],
                                mybir.ActivationFunctionType.Exp,
                                bias=bias_ap,
                                scale=2.0,
                            )
                            dest = out_d[
                                mt * P : (mt + 1) * P,
                                (c0 + a) * 512 : (c0 + b) * 512,
                            ].rearrange("p (t m) -> p t m", t=b - a)
                            nc.sync.dma_start(dest, o_sb[:, a:b, :])

    nc.finalize()
    return nc


def _get_nc():
    if "nc" not in _cache:
        _cache["nc"] = _build_bass()
    return _cache["nc"]


def _prep_core_arrays(x, prototypes, nshard):
    """Per-core host arrays: xr [66, nshard] f16, lhs [66, 512] f16, npsq."""
    x = np.ascontiguousarray(np.asarray(x, dtype=np.float32))
    prototypes = np.ascontiguousarray(np.asarray(prototypes, dtype=np.float32))

    x16 = x.astype(np.float16)
    p16 = prototypes.astype(np.float16)

    # squared norms of the QUANTIZED values (kernel computes exact RBF of
    # the fp16 inputs), split hi/lo in fp16 for the GEMM rows
    nxsq = (-0.5 * (x16.astype(np.float64) ** 2).sum(axis=1)).astype(np.float32)
    nxh = nxsq.astype(np.float16)
    nxl = (nxsq - nxh.astype(np.float32)).astype(np.float16)

    psq = (p16.astype(np.float64) ** 2).sum(axis=1)  # [512]
    npsq = np.ascontiguousarray(
        (-psq.astype(np.float32)).reshape(MT, P).T
    )  # [128, 4]

    ones = np.ones((1, M), dtype=np.float16)
    lhs = np.ascontiguousarray(
        np.concatenate([p16.T, ones, ones], axis=0)
    )  # [66, 512]

    ncores = x.shape[0] // nshard
    in_maps = []
    for s in range(ncores):
        sl = slice(s * nshard, (s + 1) * nshard)
        xr = np.empty((K1, nshard), dtype=np.float16)
        xr[:D] = x16[sl].T
        xr[D] = nxh[sl]
        xr[D + 1] = nxl[sl]
        in_maps.append({"xr": xr, "lhs": lhs, "npsq": npsq})
    return in_maps


def _prep_inputs(x, prototypes):
    return _prep_core_arrays(x, prototypes, NSHARD)


def _run(inputs, trace=False):
    from concourse.bass_utils import run_bass_kernel_spmd

    in_maps = _prep_inputs(inputs["x"], inputs["prototypes"])
    nc = _get_nc()
    res = run_bass_kernel_spmd(
        nc, in_maps, core_ids=list(range(NCORES)), trace=trace
    )
    out = np.empty((N, M), dtype=np.float32)
    for s, r in enumerate(res.results):
        # r["out"] is [512, 16384] bf16 -> [16384, 512] f32
        out[s * NSHARD : (s + 1) * NSHARD] = r["out"].astype(np.float32).T
    return out, res


def kernel(**inputs) -> np.ndarray:
    out, _ = _run(inputs, trace=False)
    return out


# revision 6
# speedup vs baseline: 1.0070x; 1.0070x over previous
"""RBF kernel layer (retrieval_knn): out = exp(-||x - p||^2) for x [131072, 64]
against 512 prototypes, distributed data-parallel over 8 NeuronCores.

v2 design ([m,n] orientation, single fp16 GEMM, bf16 output):
  out[m, n] = exp(2*S[m, n] - p_sq[m]),  S = cross - x_sq/2
computed as ONE fp16 matmul per 512-column tile with K=66:
  lhsT = [p_t(64 rows); 1; 1]  (stationary, per 128-prototype tile)
  rhs  = [x_t(64 rows); nxsq_h; nxsq_l]  (x features + fp16 hi/lo of -x_sq/2)
The exact -p_sq[m] rides in the ACTIVATE's per-partition bias AP (f32), and
scale=2.0 turns PSUM S into exp(2S - p_sq) in one pass, emitted directly as
bf16 (halves output DMA bytes vs f32; host upconverts + transposes).

Engine budget per core (16384 points, 512 protos): ScalarE exp is the
bottleneck: 65536 elem/lane / 1.2GHz + 32 ACT overheads ~= 63us. DMA moves
2.2MB in + 16.8MB out ~= 50us. PE streams 128 matmuls of 512 cols ~= 27-55us
(HAM-dependent). DVE is idle (prototypes stationary => no lhsT rebuilds).
x_sq/p_sq are computed from the QUANTIZED fp16 inputs, so the kernel is the
exact RBF of (x16, p16): error ~ 2|x-p|*q_rms, small precisely where the
output is large.
"""

import numpy as np

# Problem constants (hardcoded per harness contract; kernel.py is self-contained)
N = 131072
D = 64
M = 512
GAMMA = 1.0
NCORES = 8
NSHARD = N // NCORES  # 16384
P = 128
K1 = D + 2  # contraction: 64 x rows + 2 (-x_sq/2 hi/lo vs ones) rows
MT = M // P  # 4 prototype tiles
NT = NSHARD // 512  # 32 column chunks of 512 points
OCHUNK = 4  # PSUM banks per ACTIVATE + output DMA (double-buffered 4+4)

_cache = {}


def _build_bass(nshard=NSHARD):
    import concourse.mybir as mybir
    import concourse.tile as tile
    from concourse import bacc

    f32 = mybir.dt.float32
    f16 = mybir.dt.float16
    bf16 = mybir.dt.bfloat16

    nc = bacc.Bacc(None, target_bir_lowering=False)
    # x pre-transposed on host: rows 0..63 = features, 64/65 = -x_sq/2 hi/lo
    xr_d = nc.dram_tensor("xr", [K1, nshard], f16, kind="ExternalInput")
    # prototypes transposed: rows 0..63 = features, 64/65 = ones
    lhs_d = nc.dram_tensor("lhs", [K1, M], f16, kind="ExternalInput")
    # npsq[p, t] = -p_sq[t*128 + p] (f32, exact)
    npsq_d = nc.dram_tensor("npsq", [P, MT], f32, kind="ExternalInput")
    # output transposed: out_t[m, n]; host converts to [n, m] f32
    out_d = nc.dram_tensor("out", [M, nshard], bf16, kind="ExternalOutput")

    with tile.TileContext(nc) as tc:
        with (
            tc.tile_pool(name="singles", bufs=1) as singles,
            tc.tile_pool(name="outp", bufs=4) as outp,
            tc.tile_pool(name="ps_o", bufs=2, space="PSUM") as ps_o,
        ):
            lhs_sb = singles.tile([K1, M], f16)
            nc.sync.dma_start(lhs_sb[:], lhs_d[:])
            npsq_sb = singles.tile([P, MT], f32)
            nc.sync.dma_start(npsq_sb[:], npsq_d[:])

            # x stays resident in SBUF (2.2MB). The HWDGE queue is keyed to
            # the issuing engine, and one queue feeds descriptors too slowly
            # during the early phase -- so round-robin the input chunks over
            # three queues (vector/gpsimd are otherwise idle), with small
            # leading chunks so the first matmuls start ASAP.
            X_sb = singles.tile([K1, nshard], f16)
            # (HWDGE issuers are gpsimd/sync/scalar; scalar only issues
            # during its idle pre-ACT window, the queue itself runs async)
            qs = [nc.gpsimd, nc.scalar, nc.sync]
            pos = 0
            for i, ch in enumerate((512, 512, 1024, 2048, 2048, 2048, 4096, 4096)):
                qs[i % 3].dma_start(
                    X_sb[:, pos : pos + ch], xr_d[:, pos : pos + ch]
                )
                pos += ch
            assert pos == nshard

            for mt in range(MT):
                lhs_ap = lhs_sb[:, mt * P : (mt + 1) * P]
                bias_ap = npsq_sb[:, mt : mt + 1]
                for c in range(NT):
                    k = c % OCHUNK
                    if k == 0:
                        psum = ps_o.tile([P, OCHUNK, 512], f32, tag="psum")
                        o_sb = outp.tile([P, OCHUNK, 512], bf16, tag="o")
                    nc.tensor.matmul(
                        psum[:, k, :],
                        lhs_ap,
                        X_sb[:, c * 512 : (c + 1) * 512],
                        start=True,
                        stop=True,
                    )
                    if k == OCHUNK - 1:
                        c0 = c - (OCHUNK - 1)
                        last = mt == MT - 1 and c == NT - 1
                        # out = exp(2*S - p_sq); the very last group is split
                        # in halves so the final output DMA tail is shorter
                        splits = (
                            [(0, 2), (2, 4)] if last else [(0, OCHUNK)]
                        )
                        for a, b in splits:
                            nc.scalar.activation(
                                o_sb[:, a:b, :],
                                psum[:, a:b, :],
                                mybir.ActivationFunctionType.Exp,
                                bias=bias_ap,
                                scale=2.0,
                            )
                            dest = out_d[
                                mt * P : (mt + 1) * P,
                                (c0 + a) * 512 : (c0 + b) * 512,
                            ].rearrange("p (t m) -> p t m", t=b - a)
                            nc.sync.dma_start(dest, o_sb[:, a:b, :])

    nc.finalize()
    return nc


def _get_nc():
    if "nc" not in _cache:
        _cache["nc"] = _build_bass()
    return _cache["nc"]


def _prep_core_arrays(x, prototypes, nshard):
    """Per-core host arrays: xr [66, nshard] f16, lhs [66, 512] f16, npsq."""
    x = np.ascontiguousarray(np.asarray(x, dtype=np.float32))
    prototypes = np.ascontiguousarray(np.asarray(prototypes, dtype=np.float32))

    x16 = x.astype(np.float16)
    p16 = prototypes.astype(np.float16)

    # squared norms of the QUANTIZED values (kernel computes exact RBF of
    # the fp16 inputs), split hi/lo in fp16 for the GEMM rows
    nxsq = (-0.5 * (x16.astype(np.float64) ** 2).sum(axis=1)).astype(np.float32)
    nxh = nxsq.astype(np.float16)
    nxl = (nxsq - nxh.astype(np.float32)).astype(np.float16)

    psq = (p16.astype(np.float64) ** 2).sum(axis=1)  # [512]
    npsq = np.ascontiguousarray(
        (-psq.astype(np.float32)).reshape(MT, P).T
    )  # [128, 4]

    ones = np.ones((1, M), dtype=np.float16)
    lhs = np.ascontiguousarray(
        np.concatenate([p16.T, ones, ones], axis=0)
    )  # [66, 512]

    ncores = x.shape[0] // nshard
    in_maps = []
    for s in range(ncores):
        sl = slice(s * nshard, (s + 1) * nshard)
        xr = np.empty((K1, nshard), dtype=np.float16)
        xr[:D] = x16[sl].T
        xr[D] = nxh[sl]
        xr[D + 1] = nxl[sl]
        in_maps.append({"xr": xr, "lhs": lhs, "npsq": npsq})
    return in_maps


def _prep_inputs(x, prototypes):
    return _prep_core_arrays(x, prototypes, NSHARD)


def _run(inputs, trace=False):
    from concourse.bass_utils import run_bass_kernel_spmd

    in_maps = _prep_inputs(inputs["x"], inputs["prototypes"])
    nc = _get_nc()
    res = run_bass_kernel_spmd(
        nc, in_maps, core_ids=list(range(NCORES)), trace=trace
    )
    out = np.empty((N, M), dtype=np.float32)
    for s, r in enumerate(res.results):
        # r["out"] is [512, 16384] bf16 -> [16384, 512] f32
        out[s * NSHARD : (s + 1) * NSHARD] = r["out"].astype(np.float32).T
    return out, res


def kernel(**inputs) -> np.ndarray:
    out, _ = _run(inputs, trace=False)
    return out
